# revision 10
# baseline (speedup 1.0000x reference)
"""DropBlock (B,C,H,W)=(64,64,112,112) f32 on 8 trn2 NeuronCores.

Strategy (data-parallel over batch, 8 batches/core):
  out = x * mask[b,h,w] * (B*H*W) / sum(x * mask)

Per-core bass kernel, minimum HBM traffic (read x once, write out once):
  phase 1: stream x tiles (128 rows = 2 batches x 64 ch, free = hw chunk),
           broadcast mask rows to 128 partitions via a tiny K=2 PE matmul
           into PSUM, fused multiply+reduce (tensor_tensor_reduce) writing
           y = x*mask as fp16 into a resident SBUF buffer while
           accumulating per-partition partial sums in f32.
  all-reduce: partition_all_reduce + AllReduce collective (8 ranks, 32B).
  phase 2: scale y (fp16) by 802816/s on DVE/ACT, write f32 out.
"""

import numpy as np

B, C, H, W = 64, 64, 112, 112
HW = H * W  # 12544
N_CORES = 8
B_LOC = B // N_CORES           # 8 batches per core
R = B_LOC * C                  # 512 rows per core
N_GROUPS = R // 128            # 4 groups of 128 rows (2 batches each)
KEEP_PROB = 0.9
BLOCK_SIZE = 7
RENORM_N = float(B * H * W)    # 802816 (matches reference: B*H*W, no C)

_cache = {}


def _host_mask():
    """(64, HW) f32 mask, exactly matching the reference's RNG (threefry, key 42)."""
    if "mask" in _cache:
        return _cache["mask"]
    import jax
    import jax.numpy as jnp
    from jax import lax

    cpu = jax.devices("cpu")[0]
    with jax.default_device(cpu):
        gamma = (1.0 - KEEP_PROB) * (H**2) / (BLOCK_SIZE**2 * (H - BLOCK_SIZE + 1) ** 2)
        key = jax.random.key(42)
        init_mask = jax.random.bernoulli(key, 1.0 - gamma, (B, H, W)).astype(jnp.float32)
        k = BLOCK_SIZE // 2
        m = lax.reduce_window(
            init_mask,
            jnp.array(1.0, init_mask.dtype),
            lax.min,
            window_dimensions=(1, 2 * k, 2 * k),
            window_strides=(1, 1, 1),
            padding=((0, 0), (k - 1, k), (k - 1, k)),
        )
        mask = np.asarray(m).reshape(B, HW)
    _cache["mask"] = mask
    return mask


def _build_bass():
    if "nc" in _cache:
        return _cache["nc"]
    import concourse.bacc as bacc
    import concourse.mybir as mybir
    from concourse.bass_isa import ReduceOp
    from concourse.tile import TileContext

    f32 = mybir.dt.float32
    f16 = mybir.dt.float16
    f8 = mybir.dt.float8e4
    mult = mybir.AluOpType.mult
    add = mybir.AluOpType.add

    nc = bacc.Bacc("TRN2", num_devices=N_CORES)
    x = nc.dram_tensor("x", [R, HW], f32, kind="ExternalInput")
    mask_in = nc.dram_tensor("mask", [B_LOC, HW], f8, kind="ExternalInput")
    out = nc.dram_tensor("out", [R, HW], f32, kind="ExternalOutput")

    FJ = 2048   # x/out DMA tile free size (8 KiB rows)
    FD = 1024   # DVE op / PSUM mask tile free size (2 PSUM banks)
    FM = 512    # PE matmul moving free dim limit

    with TileContext(nc) as tc:
        with tc.tile_pool(name="resid", bufs=1) as resid, \
             tc.tile_pool(name="xin", bufs=3) as xin, \
             tc.tile_pool(name="oout", bufs=3) as oout, \
             tc.tile_pool(name="mps", bufs=3, space="PSUM") as mps, \
             tc.tile_pool(name="small", bufs=1) as small, \
             tc.tile_pool(name="dram", bufs=1, space="DRAM") as dram:

            # --- constants / persistent tiles ---
            # sel_all[:, 128g:128(g+1)] = (8,128) selector for group g:
            # column m picks mask row 2g (m<64) or 2g+1 (m>=64)
            f8np = mybir.dt.np(f8)
            sel_np = np.zeros((8, N_GROUPS * 128), dtype=f8np)
            for g in range(N_GROUPS):
                sel_np[2 * g, 128 * g : 128 * g + 64] = f8np(1.0)
                sel_np[2 * g + 1, 128 * g + 64 : 128 * (g + 1)] = f8np(1.0)
            sel_dram = nc.inline_tensor(sel_np, name="sel_const")
            sel_all = small.tile([8, N_GROUPS * 128], f8, tag="sel_all")
            nc.sync.dma_start(out=sel_all[:], in_=sel_dram[:, :])

            # per-tile partial sums land in columns of accq; reduced at the end
            accq = small.tile([128, 64], f32, tag="accq")
            nc.vector.memset(accq[:], 0.0)

            mask_sb = small.tile([B_LOC, HW], f8, tag="mask_sb")
            nc.sync.dma_start(out=mask_sb[:], in_=mask_in[:, :])

            y16 = [resid.tile([128, HW], f16, tag=f"y{g}", name=f"y{g}") for g in range(N_GROUPS)]

            # --- phase 1: y = x*mask (fp16, resident), accq[:, k] = rowsum(tile) ---
            k = 0
            for g in range(N_GROUPS):
                for j0 in range(0, HW, FJ):
                    fj = min(FJ, HW - j0)
                    xt = xin.tile([128, FJ], f32, tag="xt", name="xt")
                    nc.sync.dma_start(
                        out=xt[:, :fj], in_=x[128 * g : 128 * (g + 1), j0 : j0 + fj]
                    )
                    for h0 in range(0, fj, FD):
                        fd = min(FD, fj - h0)
                        mt = mps.tile([128, FD], f32, tag="mt", name="mt")
                        for m0 in range(0, fd, FM):
                            fm = min(FM, fd - m0)
                            nc.tensor.matmul(
                                mt[:, m0 : m0 + fm],
                                lhsT=sel_all[:, 128 * g : 128 * (g + 1)],
                                rhs=mask_sb[:, j0 + h0 + m0 : j0 + h0 + m0 + fm],
                                start=True,
                                stop=True,
                            )
                        nc.vector.scalar_tensor_tensor(
                            out=y16[g][:, j0 + h0 : j0 + h0 + fd],
                            in0=xt[:, h0 : h0 + fd],
                            scalar=1.0,
                            in1=mt[:, :fd],
                            op0=mult,
                            op1=mult,
                            accum_out=accq[:, k : k + 1],
                        )
                        k += 1
            assert k <= 64

            # --- global sum: partition reduce, 8-rank AllReduce, scale bcast ---
            acc = small.tile([128, 1], f32, tag="acc")
            nc.vector.tensor_reduce(
                out=acc[:], in_=accq[:], axis=mybir.AxisListType.X, op=add
            )
            accr = small.tile([128, 1], f32, tag="accr")
            nc.gpsimd.partition_all_reduce(accr[:], acc[:], channels=128, reduce_op=ReduceOp.add)

            s_pad = small.tile([1, 8], f32, tag="s_pad")
            nc.vector.memset(s_pad[:], 0.0)
            nc.vector.tensor_copy(out=s_pad[0:1, 0:1], in_=accr[0:1, 0:1])

            cc_in = dram.tile([1, 8], f32, tag="cc_in", name="cc_in")
            cc_out = dram.tile([1, 8], f32, tag="cc_out", name="cc_out", addr_space="Shared")
            nc.sync.dma_start(out=cc_in[:], in_=s_pad[:])
            nc.gpsimd.collective_compute(
                "AllReduce",
                mybir.AluOpType.add,
                replica_groups=[list(range(N_CORES))],
                ins=[cc_in.opt()],
                outs=[cc_out.opt()],
            )
            s_all = small.tile([1, 8], f32, tag="s_all")
            nc.sync.dma_start(out=s_all[:], in_=cc_out[:])

            rec = small.tile([1, 1], f32, tag="rec")
            nc.vector.reciprocal(out=rec[:], in_=s_all[0:1, 0:1])
            scl1 = small.tile([1, 1], f32, tag="scl1")
            nc.vector.tensor_scalar_mul(scl1[:], rec[:], RENORM_N)
            scale_sb = small.tile([128, 1], f32, tag="scale_sb")
            nc.gpsimd.partition_broadcast(scale_sb[:], scl1[:], channels=128)

            # --- phase 2: out = y16 * scale (alternate DVE / ACT) ---
            idx = 0
            for g in range(N_GROUPS):
                for j0 in range(0, HW, FJ):
                    fj = min(FJ, HW - j0)
                    ot = oout.tile([128, FJ], f32, tag="ot", name="ot")
                    if idx % 2 == 0:
                        nc.vector.tensor_scalar_mul(
                            ot[:, :fj], y16[g][:, j0 : j0 + fj], scale_sb[:]
                        )
                    else:
                        nc.scalar.activation(
                            out=ot[:, :fj],
                            in_=y16[g][:, j0 : j0 + fj],
                            func=mybir.ActivationFunctionType.Copy,
                            scale=scale_sb[:],
                        )
                    idx += 1
                    nc.sync.dma_start(
                        out=out[128 * g : 128 * (g + 1), j0 : j0 + fj], in_=ot[:, :fj]
                    )

    nc.compile()
    _cache["nc"] = nc
    return nc


def kernel(x: np.ndarray) -> np.ndarray:
    import concourse.mybir as mybir
    from concourse.bass_utils import run_bass_kernel_spmd

    x = np.ascontiguousarray(x, dtype=np.float32)
    assert x.shape == (B, C, H, W)
    mask = _host_mask()  # (64, HW) f32
    f8np = mybir.dt.np(mybir.dt.float8e4)
    mask8 = mask.astype(f8np)

    nc = _build_bass()
    in_maps = []
    for c in range(N_CORES):
        xs = x[c * B_LOC : (c + 1) * B_LOC].reshape(R, HW)
        ms = mask8[c * B_LOC : (c + 1) * B_LOC]
        in_maps.append({"x": xs, "mask": ms})

    res = run_bass_kernel_spmd(nc, in_maps, core_ids=list(range(N_CORES)))
    outs = [res.results[c]["out"].reshape(B_LOC, C, H, W) for c in range(N_CORES)]
    return np.concatenate(outs, axis=0)


# revision 15
# speedup vs baseline: 4.8722x; 4.8722x over previous
"""DropBlock (B,C,H,W)=(64,64,112,112) f32 on 8 trn2 NeuronCores.

Strategy (data-parallel over batch, 8 batches/core):
  out = x * mask[b,h,w] * (B*H*W) / sum(x * mask)

Per-core bass kernel, minimum HBM traffic (read x once, write out once):
  phase 1: stream x tiles (128 rows = 2 batches x 64 ch, free = hw chunk),
           broadcast mask rows to 128 partitions via a tiny K=8 PE matmul
           into PSUM, fused multiply+row-sum (scalar_tensor_tensor) writing
           y = x*mask as fp16 into a resident SBUF buffer while emitting
           per-tile partial sums in f32.
  all-reduce: partition_all_reduce + AllReduce collective (8 ranks, 32B).
  phase 2: scale y (fp16) by 802816/s on DVE/ACT, write f32 out.

The `repeat` parameter replicates the whole computation in one NEFF for
benchmarking (marginal time per repeat = steady-state HW time).
"""

import numpy as np

B, C, H, W = 64, 64, 112, 112
HW = H * W  # 12544
N_CORES = 8
B_LOC = B // N_CORES           # 8 batches per core
R = B_LOC * C                  # 512 rows per core
N_GROUPS = R // 128            # 4 groups of 128 rows (2 batches each)
KEEP_PROB = 0.9
BLOCK_SIZE = 7
RENORM_N = float(B * H * W)    # 802816 (matches reference: B*H*W, no C)

_cache = {}


def _host_mask():
    """(64, HW) f32 mask, exactly matching the reference's RNG (threefry, key 42)."""
    if "mask" in _cache:
        return _cache["mask"]
    import jax
    import jax.numpy as jnp
    from jax import lax

    cpu = jax.devices("cpu")[0]
    with jax.default_device(cpu):
        gamma = (1.0 - KEEP_PROB) * (H**2) / (BLOCK_SIZE**2 * (H - BLOCK_SIZE + 1) ** 2)
        key = jax.random.key(42)
        init_mask = jax.random.bernoulli(key, 1.0 - gamma, (B, H, W)).astype(jnp.float32)
        k = BLOCK_SIZE // 2
        m = lax.reduce_window(
            init_mask,
            jnp.array(1.0, init_mask.dtype),
            lax.min,
            window_dimensions=(1, 2 * k, 2 * k),
            window_strides=(1, 1, 1),
            padding=((0, 0), (k - 1, k), (k - 1, k)),
        )
        mask = np.asarray(m).reshape(B, HW)
    _cache["mask"] = mask
    return mask


def _build_bass(repeat: int = 1):
    key = ("nc", repeat)
    if key in _cache:
        return _cache[key]
    import concourse.bacc as bacc
    import concourse.mybir as mybir
    from concourse.bass_isa import ReduceOp
    from concourse.tile import TileContext

    f32 = mybir.dt.float32
    f16 = mybir.dt.float16
    f8 = mybir.dt.float8e4
    mult = mybir.AluOpType.mult
    add = mybir.AluOpType.add

    nc = bacc.Bacc("TRN2", num_devices=N_CORES)
    x = nc.dram_tensor("x", [R, HW], f32, kind="ExternalInput")
    mask_in = nc.dram_tensor("mask", [B_LOC, HW], f8, kind="ExternalInput")
    out = nc.dram_tensor("out", [R, HW], f32, kind="ExternalOutput")

    FJ = 2048   # x/out DMA tile free size (8 KiB rows)
    FD = 1024   # DVE op / PSUM mask tile free size (2 PSUM banks)
    FM = 512    # PE matmul moving free dim limit

    with TileContext(nc) as tc:
        with tc.tile_pool(name="resid", bufs=1) as resid, \
             tc.tile_pool(name="xin", bufs=3) as xin, \
             tc.tile_pool(name="oout", bufs=3) as oout, \
             tc.tile_pool(name="mps", bufs=3, space="PSUM") as mps, \
             tc.tile_pool(name="small", bufs=1) as small, \
             tc.tile_pool(name="dram", bufs=1, space="DRAM") as dram:

            # --- constants / persistent tiles ---
            # sel_all[:, 128g:128(g+1)] = (8,128) selector for group g:
            # column m picks mask row 2g (m<64) or 2g+1 (m>=64)
            f8np = mybir.dt.np(f8)
            sel_np = np.zeros((8, N_GROUPS * 128), dtype=f8np)
            for g in range(N_GROUPS):
                sel_np[2 * g, 128 * g : 128 * g + 64] = f8np(1.0)
                sel_np[2 * g + 1, 128 * g + 64 : 128 * (g + 1)] = f8np(1.0)
            sel_dram = nc.inline_tensor(sel_np, name="sel_const")
            sel_all = small.tile([8, N_GROUPS * 128], f8, tag="sel_all")
            nc.sync.dma_start(out=sel_all[:], in_=sel_dram[:, :])

            # per-tile partial sums land in columns of accq; reduced at the end
            accq = small.tile([128, 64], f32, tag="accq")
            nc.vector.memset(accq[:], 0.0)

            mask_sb = small.tile([B_LOC, HW], f8, tag="mask_sb")
            nc.sync.dma_start(out=mask_sb[:], in_=mask_in[:, :])

            y16 = [resid.tile([128, HW], f16, tag=f"y{g}", name=f"y{g}") for g in range(N_GROUPS)]

            for _rep in range(repeat):
                cc_in = dram.tile([1, 8], f32, tag=f"cc_in{_rep}", name=f"cc_in{_rep}")
                cc_out = dram.tile(
                    [1, 8], f32, tag=f"cc_out{_rep}", name=f"cc_out{_rep}", addr_space="Shared"
                )
                # --- phase 1 ---
                k = 0
                for g in range(N_GROUPS):
                    for j0 in range(0, HW, FJ):
                        fj = min(FJ, HW - j0)
                        xt = xin.tile([128, FJ], f32, tag="xt", name="xt")
                        nc.sync.dma_start(
                            out=xt[:, :fj], in_=x[128 * g : 128 * (g + 1), j0 : j0 + fj]
                        )
                        for h0 in range(0, fj, FD):
                            fd = min(FD, fj - h0)
                            mt = mps.tile([128, FD], f32, tag="mt", name="mt")
                            for m0 in range(0, fd, FM):
                                fm = min(FM, fd - m0)
                                nc.tensor.matmul(
                                    mt[:, m0 : m0 + fm],
                                    lhsT=sel_all[:, 128 * g : 128 * (g + 1)],
                                    rhs=mask_sb[:, j0 + h0 + m0 : j0 + h0 + m0 + fm],
                                    start=True,
                                    stop=True,
                                )
                            nc.vector.scalar_tensor_tensor(
                                out=y16[g][:, j0 + h0 : j0 + h0 + fd],
                                in0=xt[:, h0 : h0 + fd],
                                scalar=1.0,
                                in1=mt[:, :fd],
                                op0=mult,
                                op1=mult,
                                accum_out=accq[:, k : k + 1],
                            )
                            k += 1
                assert k <= 64

                # --- global sum: partition reduce, 8-rank AllReduce, scale bcast ---
                acc = small.tile([128, 1], f32, tag="acc", name="acc")
                nc.vector.tensor_reduce(
                    out=acc[:], in_=accq[:], axis=mybir.AxisListType.X, op=add
                )
                accr = small.tile([128, 1], f32, tag="accr", name="accr")
                nc.gpsimd.partition_all_reduce(accr[:], acc[:], channels=128, reduce_op=ReduceOp.add)

                s_pad = small.tile([1, 8], f32, tag="s_pad", name="s_pad")
                nc.vector.memset(s_pad[:], 0.0)
                nc.vector.tensor_copy(out=s_pad[0:1, 0:1], in_=accr[0:1, 0:1])

                nc.sync.dma_start(out=cc_in[:], in_=s_pad[:])
                nc.gpsimd.collective_compute(
                    "AllReduce",
                    mybir.AluOpType.add,
                    replica_groups=[list(range(N_CORES))],
                    ins=[cc_in.opt()],
                    outs=[cc_out.opt()],
                )
                s_all = small.tile([1, 8], f32, tag="s_all", name="s_all")
                nc.sync.dma_start(out=s_all[:], in_=cc_out[:])

                rec = small.tile([1, 1], f32, tag="rec", name="rec")
                nc.vector.reciprocal(out=rec[:], in_=s_all[0:1, 0:1])
                scl1 = small.tile([1, 1], f32, tag="scl1", name="scl1")
                nc.vector.tensor_scalar_mul(scl1[:], rec[:], RENORM_N)
                scale_sb = small.tile([128, 1], f32, tag="scale_sb", name="scale_sb")
                nc.gpsimd.partition_broadcast(scale_sb[:], scl1[:], channels=128)

                # --- phase 2: out = y16 * scale (alternate DVE / ACT) ---
                idx = 0
                for g in range(N_GROUPS):
                    for j0 in range(0, HW, FJ):
                        fj = min(FJ, HW - j0)
                        ot = oout.tile([128, FJ], f32, tag="ot", name="ot")
                        if idx % 2 == 0:
                            nc.vector.tensor_scalar_mul(
                                ot[:, :fj], y16[g][:, j0 : j0 + fj], scale_sb[:]
                            )
                        else:
                            nc.scalar.activation(
                                out=ot[:, :fj],
                                in_=y16[g][:, j0 : j0 + fj],
                                func=mybir.ActivationFunctionType.Copy,
                                scale=scale_sb[:],
                            )
                        idx += 1
                        nc.sync.dma_start(
                            out=out[128 * g : 128 * (g + 1), j0 : j0 + fj], in_=ot[:, :fj]
                        )

    nc.compile()
    _cache[key] = nc
    return nc


def kernel(x: np.ndarray) -> np.ndarray:
    import concourse.mybir as mybir
    from concourse.bass_utils import run_bass_kernel_spmd

    x = np.ascontiguousarray(x, dtype=np.float32)
    assert x.shape == (B, C, H, W)
    mask = _host_mask()  # (64, HW) f32
    f8np = mybir.dt.np(mybir.dt.float8e4)
    mask8 = mask.astype(f8np)

    nc = _build_bass()
    in_maps = []
    for c in range(N_CORES):
        xs = x[c * B_LOC : (c + 1) * B_LOC].reshape(R, HW)
        ms = mask8[c * B_LOC : (c + 1) * B_LOC]
        in_maps.append({"x": xs, "mask": ms})

    res = run_bass_kernel_spmd(nc, in_maps, core_ids=list(range(N_CORES)))
    outs = [res.results[c]["out"].reshape(B_LOC, C, H, W) for c in range(N_CORES)]
    return np.concatenate(outs, axis=0)


# revision 18
# speedup vs baseline: 21.1293x; 4.3367x over previous
"""DropBlock (B,C,H,W)=(64,64,112,112) f32 on 8 trn2 NeuronCores.

Strategy (data-parallel over batch, 8 batches/core):
  out = x * mask[b,h,w] * (B*H*W) / sum(x * mask)

Per-core bass kernel, minimum HBM traffic (read x once, write out once):
  phase 1: stream x tiles (128 rows = 2 batches x 64 ch, free = hw chunk),
           broadcast mask rows to 128 partitions via a tiny K=8 PE matmul
           into PSUM, fused multiply+row-sum (scalar_tensor_tensor) writing
           y = x*mask as fp16 into a resident SBUF buffer while emitting
           per-tile partial sums in f32.
  all-reduce: partition_all_reduce + AllReduce collective (8 ranks, 32B).
  phase 2: scale y (fp16) by 802816/s on DVE/ACT, write f32 out.

The `repeat` parameter replicates the whole computation in one NEFF for
benchmarking (marginal time per repeat = steady-state HW time).
"""

import numpy as np

B, C, H, W = 64, 64, 112, 112
HW = H * W  # 12544
N_CORES = 8
B_LOC = B // N_CORES           # 8 batches per core
R = B_LOC * C                  # 512 rows per core
N_GROUPS = R // 128            # 4 groups of 128 rows (2 batches each)
KEEP_PROB = 0.9
BLOCK_SIZE = 7
RENORM_N = float(B * H * W)    # 802816 (matches reference: B*H*W, no C)

_cache = {}


def _host_mask():
    """(64, HW) f32 mask, exactly matching the reference's RNG (threefry, key 42)."""
    if "mask" in _cache:
        return _cache["mask"]
    import jax
    import jax.numpy as jnp
    from jax import lax

    cpu = jax.devices("cpu")[0]
    with jax.default_device(cpu):
        gamma = (1.0 - KEEP_PROB) * (H**2) / (BLOCK_SIZE**2 * (H - BLOCK_SIZE + 1) ** 2)
        key = jax.random.key(42)
        init_mask = jax.random.bernoulli(key, 1.0 - gamma, (B, H, W)).astype(jnp.float32)
        k = BLOCK_SIZE // 2
        m = lax.reduce_window(
            init_mask,
            jnp.array(1.0, init_mask.dtype),
            lax.min,
            window_dimensions=(1, 2 * k, 2 * k),
            window_strides=(1, 1, 1),
            padding=((0, 0), (k - 1, k), (k - 1, k)),
        )
        mask = np.asarray(m).reshape(B, HW)
    _cache["mask"] = mask
    return mask


def _build_bass(repeat: int = 1):
    key = ("nc", repeat)
    if key in _cache:
        return _cache[key]
    import concourse.bacc as bacc
    import concourse.mybir as mybir
    from concourse.bass_isa import ReduceOp
    from concourse.tile import TileContext

    f32 = mybir.dt.float32
    f16 = mybir.dt.float16
    f8 = mybir.dt.float8e4
    mult = mybir.AluOpType.mult
    add = mybir.AluOpType.add

    nc = bacc.Bacc("TRN2", num_devices=N_CORES)
    x = nc.dram_tensor("x", [R, HW], f32, kind="ExternalInput")
    mask_in = nc.dram_tensor("mask", [B_LOC, HW], f8, kind="ExternalInput")
    out = nc.dram_tensor("out", [R, HW], f32, kind="ExternalOutput")

    FJ = 2048   # x/out DMA tile free size (8 KiB rows)
    FD = 1024   # DVE op / PSUM mask tile free size (2 PSUM banks)
    FM = 512    # PE matmul moving free dim limit

    with TileContext(nc) as tc:
        with tc.tile_pool(name="resid", bufs=1) as resid, \
             tc.tile_pool(name="xin", bufs=3) as xin, \
             tc.tile_pool(name="oout", bufs=3) as oout, \
             tc.tile_pool(name="mps", bufs=3, space="PSUM") as mps, \
             tc.tile_pool(name="small", bufs=1) as small, \
             tc.tile_pool(name="dram", bufs=1, space="DRAM") as dram:

            # --- constants / persistent tiles ---
            # sel_all[:, 128g:128(g+1)] = (8,128) selector for group g:
            # column m picks mask row 2g (m<64) or 2g+1 (m>=64)
            f8np = mybir.dt.np(f8)
            sel_np = np.zeros((8, N_GROUPS * 128), dtype=f8np)
            for g in range(N_GROUPS):
                sel_np[2 * g, 128 * g : 128 * g + 64] = f8np(1.0)
                sel_np[2 * g + 1, 128 * g + 64 : 128 * (g + 1)] = f8np(1.0)
            sel_dram = nc.inline_tensor(sel_np, name="sel_const")
            sel_all = small.tile([8, N_GROUPS * 128], f8, tag="sel_all")
            nc.sync.dma_start(out=sel_all[:], in_=sel_dram[:, :])

            # per-tile partial sums land in columns of accq; reduced at the end
            accq = small.tile([128, 64], f32, tag="accq")
            nc.vector.memset(accq[:], 0.0)

            mask_sb = small.tile([B_LOC, HW], f8, tag="mask_sb")
            nc.sync.dma_start(out=mask_sb[:], in_=mask_in[:, :])

            y16 = [resid.tile([128, HW], f16, tag=f"y{g}", name=f"y{g}") for g in range(N_GROUPS)]

            for _rep in range(repeat):
                cc_in = dram.tile([1, 8], f32, tag=f"cc_in{_rep}", name=f"cc_in{_rep}")
                cc_out = dram.tile(
                    [8, 8], f32, tag=f"cc_out{_rep}", name=f"cc_out{_rep}", addr_space="Shared"
                )
                # --- phase 1 ---
                k = 0
                for g in range(N_GROUPS):
                    for j0 in range(0, HW, FJ):
                        fj = min(FJ, HW - j0)
                        xt = xin.tile([128, FJ], f32, tag="xt", name="xt")
                        nc.sync.dma_start(
                            out=xt[:, :fj], in_=x[128 * g : 128 * (g + 1), j0 : j0 + fj]
                        )
                        for h0 in range(0, fj, FD):
                            fd = min(FD, fj - h0)
                            mt = mps.tile([128, FD], f32, tag="mt", name="mt")
                            for m0 in range(0, fd, FM):
                                fm = min(FM, fd - m0)
                                nc.tensor.matmul(
                                    mt[:, m0 : m0 + fm],
                                    lhsT=sel_all[:, 128 * g : 128 * (g + 1)],
                                    rhs=mask_sb[:, j0 + h0 + m0 : j0 + h0 + m0 + fm],
                                    start=True,
                                    stop=True,
                                )
                            nc.vector.scalar_tensor_tensor(
                                out=y16[g][:, j0 + h0 : j0 + h0 + fd],
                                in0=xt[:, h0 : h0 + fd],
                                scalar=1.0,
                                in1=mt[:, :fd],
                                op0=mult,
                                op1=mult,
                                accum_out=accq[:, k : k + 1],
                            )
                            k += 1
                assert k <= 64

                # --- global sum: partition reduce, 8-rank AllGather, scale bcast ---
                acc = small.tile([128, 1], f32, tag="acc", name="acc")
                nc.vector.tensor_reduce(
                    out=acc[:], in_=accq[:], axis=mybir.AxisListType.X, op=add
                )
                accr = small.tile([128, 1], f32, tag="accr", name="accr")
                nc.gpsimd.partition_all_reduce(accr[:], acc[:], channels=128, reduce_op=ReduceOp.add)

                s_pad = small.tile([1, 8], f32, tag="s_pad", name="s_pad")
                nc.vector.memset(s_pad[:], 0.0)
                nc.vector.tensor_copy(out=s_pad[0:1, 0:1], in_=accr[0:1, 0:1])

                nc.sync.dma_start(out=cc_in[:], in_=s_pad[:])
                # AllGather (floor ~4.6us vs AllReduce ~9.7us): out (8,8) on partition axis
                nc.gpsimd.collective_compute(
                    "AllGather",
                    mybir.AluOpType.bypass,
                    replica_groups=[list(range(N_CORES))],
                    ins=[cc_in.opt()],
                    outs=[cc_out.opt()],
                )
                # broadcast the 8 rank sums to all 128 partitions via stride-0 DMA
                s_all = small.tile([128, 8], f32, tag="s_all", name="s_all")
                nc.sync.dma_start(
                    out=s_all[:],
                    in_=cc_out[:, 0:1].transpose([1, 0]).to_broadcast((128, 8)),
                )
                ssum = small.tile([128, 1], f32, tag="ssum", name="ssum")
                nc.vector.tensor_reduce(
                    out=ssum[:], in_=s_all[:], axis=mybir.AxisListType.X, op=add
                )
                rec = small.tile([128, 1], f32, tag="rec", name="rec")
                nc.vector.reciprocal(out=rec[:], in_=ssum[:])
                scale_sb = small.tile([128, 1], f32, tag="scale_sb", name="scale_sb")
                nc.vector.tensor_scalar_mul(scale_sb[:], rec[:], RENORM_N)

                # --- phase 2: out = y16 * scale (alternate DVE / ACT) ---
                idx = 0
                for g in range(N_GROUPS):
                    for j0 in range(0, HW, FJ):
                        fj = min(FJ, HW - j0)
                        ot = oout.tile([128, FJ], f32, tag="ot", name="ot")
                        if idx % 2 == 0:
                            nc.vector.tensor_scalar_mul(
                                ot[:, :fj], y16[g][:, j0 : j0 + fj], scale_sb[:]
                            )
                        else:
                            nc.scalar.activation(
                                out=ot[:, :fj],
                                in_=y16[g][:, j0 : j0 + fj],
                                func=mybir.ActivationFunctionType.Copy,
                                scale=scale_sb[:],
                            )
                        idx += 1
                        nc.sync.dma_start(
                            out=out[128 * g : 128 * (g + 1), j0 : j0 + fj], in_=ot[:, :fj]
                        )

    nc.compile()
    _cache[key] = nc
    return nc


def kernel(x: np.ndarray) -> np.ndarray:
    import concourse.mybir as mybir
    from concourse.bass_utils import run_bass_kernel_spmd

    x = np.ascontiguousarray(x, dtype=np.float32)
    assert x.shape == (B, C, H, W)
    mask = _host_mask()  # (64, HW) f32
    f8np = mybir.dt.np(mybir.dt.float8e4)
    mask8 = mask.astype(f8np)

    nc = _build_bass()
    in_maps = []
    for c in range(N_CORES):
        xs = x[c * B_LOC : (c + 1) * B_LOC].reshape(R, HW)
        ms = mask8[c * B_LOC : (c + 1) * B_LOC]
        in_maps.append({"x": xs, "mask": ms})

    res = run_bass_kernel_spmd(nc, in_maps, core_ids=list(range(N_CORES)))
    outs = [res.results[c]["out"].reshape(B_LOC, C, H, W) for c in range(N_CORES)]
    return np.concatenate(outs, axis=0)
